# revision 5
# baseline (speedup 1.0000x reference)
"""Causal attention kernel for TRN2, 8 NeuronCores.

Problem: B=4, S=2048, D=1024 single-head causal attention, scale 1/sqrt(64).
  out = softmax_causal((x@Wq+bq) @ (x@Wk+bk)^T / 8) @ (x@Wv+bv) @ Wo + bo

Sharding: 2 cores per batch. Within a pair, query blocks (128 rows each,
16 per batch) are split A: {15,14,13,12,3,2,1,0} / B: {11..4} so causal work
balances (68 key-blocks each). SPMD requires one program for all cores, so
every core runs a uniform per-slot key-block schedule NKP=[16,16,14,14,8,7,6,6]
(87 blocks, +28% over ideal); the last 6 key-blocks of each slot get
host-provided mask tiles (0 = visible, causal triangle on the diagonal block,
-1e30 = beyond this core's causal extent or padding).

Math identities used (host-side folds):
  - bk drops entirely: (q+bq)·(k+bk) = (q+bq)·k + rowconst -> softmax invariant.
  - bv folds into output bias: P @ (V + 1·bv) @ Wo = P@V@Wo + bv@Wo (P rows sum 1).
    So bop = bo + bv @ Wo, V-projection runs biasless.
  - softmax without max-subtraction (scores bounded ~±10, exp safe in fp32);
    normalization by 1/Z folded into the attn PSUM->SBUF copy (per-partition scale).

Matmuls in float32r (fp32 storage, reduced-precision multiply, full PE rate at
moving-dim >= 256, ~1e-3 rel err) except P@V which runs bf16 (P in [0,1], V
rounded; SBUF pressure forced V to 2 bytes).
"""
import sys
sys.path.insert(0, "/opt/trn_rl_repo")

import numpy as np
from contextlib import ExitStack

import concourse.bacc as bacc
import concourse.mybir as mybir
import concourse.tile as tile
from concourse.bass_utils import run_bass_kernel_spmd
from concourse.masks import make_identity

F32 = mybir.dt.float32
F32R = mybir.dt.float32r
BF16 = mybir.dt.bfloat16
EXP = mybir.ActivationFunctionType.Exp
IDENT = mybir.ActivationFunctionType.Identity

B, S, D = 4, 2048, 1024
NB = S // 128            # 16 key/query blocks per batch
QLOC = 1024              # queries per core
SCHED_A = [15, 14, 13, 12, 3, 2, 1, 0]
SCHED_B = [11, 10, 9, 8, 7, 6, 5, 4]
NKP = [16, 16, 14, 14, 8, 7, 6, 6]   # uniform key-blocks per slot
WIN = 6                               # masked window (last WIN blocks of each slot)
MASKVAL = -1e30

_NC_CACHE = {}


def _chunks(nblk):
    """Split nblk*128 key columns into matmul chunks of width <=512, >=256."""
    total = nblk * 128
    out = []
    c0 = 0
    while c0 < total:
        cw = min(512, total - c0)
        out.append((c0, cw))
        c0 += cw
    return out


def build_nc():
    nc = bacc.Bacc("TRN2", target_bir_lowering=False, debug=False, num_devices=8)

    xt = nc.dram_tensor("xt", [D, S], F32R, kind="ExternalInput").ap()        # x^T (this batch)
    xtq = nc.dram_tensor("xtq", [D, QLOC], F32R, kind="ExternalInput").ap()   # x^T cols of my queries
    wq = nc.dram_tensor("wq", [D, D], F32R, kind="ExternalInput").ap()
    wk = nc.dram_tensor("wk", [D, D], F32R, kind="ExternalInput").ap()
    wv = nc.dram_tensor("wv", [D, D], F32R, kind="ExternalInput").ap()
    wo = nc.dram_tensor("wo", [D, D], F32R, kind="ExternalInput").ap()
    bq = nc.dram_tensor("bq", [D], F32, kind="ExternalInput").ap()
    bbc = nc.dram_tensor("bbc", [128, D], F32, kind="ExternalInput").ap()     # bop broadcast to 128 rows
    msk = nc.dram_tensor("msk", [8, 128, WIN * 128], F32, kind="ExternalInput").ap()
    outd = nc.dram_tensor("outd", [QLOC, D], F32, kind="ExternalOutput").ap()

    with tile.TileContext(nc) as tc, ExitStack() as ctx:
        # ---- pools (per-partition bytes in comments; ~208KB available)
        kt_p = ctx.enter_context(tc.tile_pool(name="kt", bufs=1))       # 8 tags x 8KB = 64
        v_p = ctx.enter_context(tc.tile_pool(name="v", bufs=1))         # 16 tags x 2KB = 32 (bf16)
        qt_p = ctx.enter_context(tc.tile_pool(name="qt", bufs=1))       # 8 tags x 4KB = 32
        w_p = ctx.enter_context(tc.tile_pool(name="w", bufs=8))         # 8 x 4KB = 32 (Wk/Wv/Wq/Wo rotate)
        xs_p = ctx.enter_context(tc.tile_pool(name="xs", bufs=16))      # 16 x 1KB = 16
        small_p = ctx.enter_context(tc.tile_pool(name="small", bufs=2)) # maskt 6 + pch 4 + osb 4 = 14
        pt_p = ctx.enter_context(tc.tile_pool(name="pt", bufs=4))       # 4 x 0.25KB = 1 (bf16)
        att_p = ctx.enter_context(tc.tile_pool(name="att", bufs=2))     # 2 x 4KB = 8
        atT_p = ctx.enter_context(tc.tile_pool(name="atT", bufs=10))    # 10 x 0.5KB = 5
        const_p = ctx.enter_context(tc.tile_pool(name="const", bufs=1)) # ~5
        zp_p = ctx.enter_context(tc.tile_pool(name="zp", bufs=4))       # ~0.1
        ps512 = ctx.enter_context(tc.tile_pool(name="ps512", bufs=2, space="PSUM"))
        psatt = ctx.enter_context(tc.tile_pool(name="psatt", bufs=2, space="PSUM"))
        pstr = ctx.enter_context(tc.tile_pool(name="pstr", bufs=2, space="PSUM"))

        # ---- constants
        ident_f = const_p.tile([128, 128], F32, tag="ident_f")
        make_identity(nc, ident_f[:])
        ident = const_p.tile([128, 128], F32R, tag="ident_r")
        nc.scalar.copy(ident[:], ident_f[:])
        bias_bc = const_p.tile([128, D], F32, tag="bias_bc")
        nc.sync.dma_start(bias_bc[:], bbc)
        bq_t = []
        for ec in range(8):
            t = const_p.tile([128, 1], F32, tag=f"bq{ec}", name=f"bqt{ec}")
            nc.sync.dma_start(t[:], bq[ec * 128:(ec + 1) * 128])
            bq_t.append(t)

        def load_w(src):
            ws = []
            for dc in range(8):
                t = w_p.tile([128, D], F32R, tag="wmat", name="wmat")
                nc.sync.dma_start(t[:], src[dc * 128:(dc + 1) * 128, :])
                ws.append(t)
            return ws

        def load_xs(src, c0):
            xsb = []
            for dc in range(8):
                t = xs_p.tile([128, 256], F32R, tag="xs", name="xs")
                nc.sync.dma_start(t[:], src[dc * 128:(dc + 1) * 128, c0:c0 + 256])
                xsb.append(t)
            return xsb

        # ---- K projection: KT[ec] = (x @ Wk)^T rows, [128 e, 2048 s]
        kt = [kt_p.tile([128, S], F32R, tag=f"kt{ec}", name=f"kt{ec}") for ec in range(8)]
        wk_t = load_w(wk)
        for sb in range(8):           # 8 s-blocks of 256
            xsb = load_xs(xt, sb * 256)
            for ec in range(8):
                ps = ps512.tile([128, 512], F32, tag="ps512", name="ps512t")
                for dc in range(8):
                    nc.tensor.matmul(ps[:, 0:256], wk_t[dc][:, ec * 128:(ec + 1) * 128],
                                     xsb[dc][:], start=(dc == 0), stop=(dc == 7))
                nc.scalar.copy(kt[ec][:, sb * 256:(sb + 1) * 256], ps[:, 0:256])

        # ---- V projection (bf16 storage): V[kb] = x @ Wv, [128 s, 1024 e]
        v = [v_p.tile([128, D], BF16, tag=f"v{kb}", name=f"v{kb}") for kb in range(NB)]
        wv_t = load_w(wv)
        for sb in range(8):
            xsb = load_xs(xt, sb * 256)
            for kq in range(2):       # 128-blocks inside the 256 s-block
                kb = sb * 2 + kq
                for eb in range(2):
                    ps = ps512.tile([128, 512], F32, tag="ps512", name="ps512t")
                    for dc in range(8):
                        nc.tensor.matmul(ps[:], xsb[dc][:, kq * 128:(kq + 1) * 128],
                                         wv_t[dc][:, eb * 512:(eb + 1) * 512],
                                         start=(dc == 0), stop=(dc == 7))
                    nc.scalar.copy(v[kb][:, eb * 512:(eb + 1) * 512], ps[:])

        # ---- Q projection (+bq): QT[ec] = (xq @ Wq + bq)^T, [128 e, 1024 q]
        qt = [qt_p.tile([128, QLOC], F32R, tag=f"qt{ec}", name=f"qt{ec}") for ec in range(8)]
        wq_t = load_w(wq)
        for qb in range(4):
            xsb = load_xs(xtq, qb * 256)
            for ec in range(8):
                ps = ps512.tile([128, 512], F32, tag="ps512", name="ps512t")
                for dc in range(8):
                    nc.tensor.matmul(ps[:, 0:256], wq_t[dc][:, ec * 128:(ec + 1) * 128],
                                     xsb[dc][:], start=(dc == 0), stop=(dc == 7))
                nc.scalar.activation(qt[ec][:, qb * 256:(qb + 1) * 256], ps[:, 0:256],
                                     IDENT, bias=bq_t[ec][:])

        # ---- Wo resident (reuses the 8 wmat slots after Wq's last read)
        wo_t = load_w(wo)

        # ---- attention slots
        for j in range(8):
            nkp = NKP[j]
            ch = _chunks(nkp)
            maskt = small_p.tile([128, WIN * 128], F32, tag="maskt", name="maskt", bufs=1)
            nc.sync.dma_start(maskt[:], msk[j, :, :])
            zparts = zp_p.tile([128, 4], F32, tag="zparts", name="zparts")
            att_ps = psatt.tile([128, D], F32, tag="psatt", name="psattt")

            for ci, (c0, cw) in enumerate(ch):
                sc = ps512.tile([128, 512], F32, tag="ps512", name="ps512t")
                for ec in range(8):
                    nc.tensor.matmul(sc[:, 0:cw], qt[ec][:, j * 128:(j + 1) * 128],
                                     kt[ec][:, c0:c0 + cw],
                                     start=(ec == 0), stop=(ec == 7))
                for w in range(WIN):
                    boff = (nkp - WIN + w) * 128
                    if c0 <= boff < c0 + cw:
                        nc.vector.tensor_add(sc[:, boff - c0:boff - c0 + 128],
                                             sc[:, boff - c0:boff - c0 + 128],
                                             maskt[:, w * 128:(w + 1) * 128])
                pch = small_p.tile([128, 512], F32R, tag="pch", name="pch")
                nc.scalar.activation(pch[:, 0:cw], sc[:, 0:cw], EXP, scale=0.125,
                                     accum_out=zparts[:, ci:ci + 1])
                for bi in range(cw // 128):
                    kb = c0 // 128 + bi
                    tr = pstr.tile([128, 128], F32R, tag="pstr", name="pstrt")
                    nc.tensor.transpose(tr[:], pch[:, bi * 128:(bi + 1) * 128], ident[:])
                    ptb = pt_p.tile([128, 128], BF16, tag="ptb", name="ptb")
                    nc.scalar.copy(ptb[:], tr[:])
                    for eb in range(2):
                        nc.tensor.matmul(att_ps[:, eb * 512:(eb + 1) * 512], ptb[:],
                                         v[kb][:, eb * 512:(eb + 1) * 512],
                                         start=(kb == 0), stop=(kb == nkp - 1))

            z = zp_p.tile([128, 1], F32, tag="z", name="zt")
            nc.vector.reduce_sum(z[:], zparts[:, 0:len(ch)], axis=mybir.AxisListType.X)
            rz = zp_p.tile([128, 1], F32, tag="rz", name="rzt")
            nc.vector.reciprocal(rz[:], z[:])
            att_sb = att_p.tile([128, D], F32R, tag="att_sb", name="att_sb")
            nc.scalar.mul(att_sb[:], att_ps[:], rz[:])

            atT = []
            for ec in range(8):
                tr = pstr.tile([128, 128], F32R, tag="pstr", name="pstrt")
                nc.tensor.transpose(tr[:], att_sb[:, ec * 128:(ec + 1) * 128], ident[:])
                t = atT_p.tile([128, 128], F32R, tag="atT", name="atTt")
                nc.scalar.copy(t[:], tr[:])
                atT.append(t)

            for eb in range(2):
                ops = ps512.tile([128, 512], F32, tag="ps512", name="ps512t")
                for ec in range(8):
                    nc.tensor.matmul(ops[:], atT[ec][:], wo_t[ec][:, eb * 512:(eb + 1) * 512],
                                     start=(ec == 0), stop=(ec == 7))
                osb = small_p.tile([128, 512], F32, tag="osb", name="osb")
                nc.vector.tensor_add(osb[:], ops[:], bias_bc[:, eb * 512:(eb + 1) * 512])
                nc.sync.dma_start(outd[j * 128:(j + 1) * 128, eb * 512:(eb + 1) * 512], osb[:])

    nc.compile()
    return nc


def _host_prep(x, Wq, bq, Wk, bk, Wv, bv, Wo, bo):
    """Build the 8 per-core input maps."""
    bop = (bo.astype(np.float64) + bv.astype(np.float64) @ Wo.astype(np.float64)).astype(np.float32)
    bbc = np.ascontiguousarray(np.broadcast_to(bop[None, :], (128, D)))
    tri = np.triu(np.full((128, 128), MASKVAL, np.float32), k=1)  # strictly-above-diag masked
    full = np.full((128, 128), MASKVAL, np.float32)
    zero = np.zeros((128, 128), np.float32)

    in_maps = []
    for core in range(8):
        b = core // 2
        sched = SCHED_A if core % 2 == 0 else SCHED_B
        xtb = np.ascontiguousarray(x[b].T)                       # [D, S]
        xtq = np.ascontiguousarray(
            np.concatenate([xtb[:, g * 128:(g + 1) * 128] for g in sched], axis=1))
        masks = np.empty((8, 128, WIN * 128), np.float32)
        for j, g in enumerate(sched):
            for w in range(WIN):
                kb = NKP[j] - WIN + w
                if kb < g:
                    m = zero
                elif kb == g:
                    m = tri
                else:
                    m = full
                masks[j, :, w * 128:(w + 1) * 128] = m
        in_maps.append({
            "xt": xtb, "xtq": xtq, "wq": Wq, "wk": Wk, "wv": Wv, "wo": Wo,
            "bq": bq, "bbc": bbc, "msk": masks,
        })
    return in_maps


def kernel(x, Wq, bq, Wk, bk, Wv, bv, Wo, bo, _trace=False):
    x = np.asarray(x, np.float32)
    args = [np.asarray(a, np.float32) for a in (Wq, bq, Wk, bk, Wv, bv, Wo, bo)]
    Wq, bq, Wk, bk, Wv, bv, Wo, bo = args

    if "nc" not in _NC_CACHE:
        _NC_CACHE["nc"] = build_nc()
    nc = _NC_CACHE["nc"]

    in_maps = _host_prep(x, Wq, bq, Wk, bk, Wv, bv, Wo, bo)
    res = run_bass_kernel_spmd(nc, in_maps, core_ids=list(range(8)), trace=_trace)
    _NC_CACHE["last_results"] = res

    out = np.empty((B, S, D), np.float32)
    for core in range(8):
        b = core // 2
        sched = SCHED_A if core % 2 == 0 else SCHED_B
        o = res.results[core]["outd"]                            # [QLOC, D]
        for j, g in enumerate(sched):
            out[b, g * 128:(g + 1) * 128, :] = o[j * 128:(j + 1) * 128, :]
    return out


# revision 6
# speedup vs baseline: 7.2369x; 7.2369x over previous
"""Causal attention kernel for TRN2, 8 NeuronCores.

Problem: B=4, S=2048, D=1024 single-head causal attention, scale 1/sqrt(64).
  out = softmax_causal((x@Wq+bq) @ (x@Wk+bk)^T / 8) @ (x@Wv+bv) @ Wo + bo

Sharding: 2 cores per batch. Within a pair, query blocks (128 rows each,
16 per batch) are split A: {15,14,13,12,3,2,1,0} / B: {11..4} so causal work
balances (68 key-blocks each). SPMD requires one program for all cores, so
every core runs a uniform per-slot key-block schedule NKP=[16,16,14,14,8,7,6,6]
(87 blocks, +28% over ideal); the last 6 key-blocks of each slot get
host-provided mask tiles (0 = visible, causal triangle on the diagonal block,
-1e30 = beyond this core's causal extent or padding).

Math identities used (host-side folds):
  - bk drops entirely: (q+bq)·(k+bk) = (q+bq)·k + rowconst -> softmax invariant.
  - bv folds into output bias: P @ (V + 1·bv) @ Wo = P@V@Wo + bv@Wo (P rows sum 1).
    So bop = bo + bv @ Wo, V-projection runs biasless.
  - softmax without max-subtraction (scores bounded ~±10, exp safe in fp32);
    normalization by 1/Z folded into the attn PSUM->SBUF copy (per-partition scale).

Matmuls in float32r (fp32 storage, reduced-precision multiply, full PE rate at
moving-dim >= 256, ~1e-3 rel err) except P@V which runs bf16 (P in [0,1], V
rounded; SBUF pressure forced V to 2 bytes).
"""
import sys
sys.path.insert(0, "/opt/trn_rl_repo")

import numpy as np
from contextlib import ExitStack

import concourse.bacc as bacc
import concourse.mybir as mybir
import concourse.tile as tile
from concourse.bass_utils import run_bass_kernel_spmd
from concourse.masks import make_identity

F32 = mybir.dt.float32
F32R = mybir.dt.float32r
BF16 = mybir.dt.bfloat16
EXP = mybir.ActivationFunctionType.Exp
IDENT = mybir.ActivationFunctionType.Identity

B, S, D = 4, 2048, 1024
NB = S // 128            # 16 key/query blocks per batch
QLOC = 1024              # queries per core
SCHED_A = [15, 14, 13, 12, 3, 2, 1, 0]
SCHED_B = [11, 10, 9, 8, 7, 6, 5, 4]
NKP = [16, 16, 14, 14, 8, 7, 6, 6]   # uniform key-blocks per slot
WIN = 6                               # masked window (last WIN blocks of each slot)
MASKVAL = -1e30

_NC_CACHE = {}


def _chunks(nblk):
    """Split nblk*128 key columns into matmul chunks of width <=512, >=256."""
    total = nblk * 128
    out = []
    c0 = 0
    while c0 < total:
        cw = min(512, total - c0)
        out.append((c0, cw))
        c0 += cw
    return out


def build_nc():
    nc = bacc.Bacc("TRN2", target_bir_lowering=False, debug=False, num_devices=8)

    xt = nc.dram_tensor("xt", [D, S], F32R, kind="ExternalInput").ap()        # x^T (this batch)
    xtq = nc.dram_tensor("xtq", [D, QLOC], F32R, kind="ExternalInput").ap()   # x^T cols of my queries
    wq = nc.dram_tensor("wq", [D, D], F32R, kind="ExternalInput").ap()
    wk = nc.dram_tensor("wk", [D, D], F32R, kind="ExternalInput").ap()
    wv = nc.dram_tensor("wv", [D, D], F32R, kind="ExternalInput").ap()
    wo = nc.dram_tensor("wo", [D, D], F32R, kind="ExternalInput").ap()
    bq = nc.dram_tensor("bq", [D], F32, kind="ExternalInput").ap()
    bbc = nc.dram_tensor("bbc", [128, D], F32, kind="ExternalInput").ap()     # bop broadcast to 128 rows
    msk = nc.dram_tensor("msk", [8, 128, WIN * 128], F32, kind="ExternalInput").ap()
    outd = nc.dram_tensor("outd", [QLOC, D], F32, kind="ExternalOutput").ap()

    with tile.TileContext(nc) as tc, ExitStack() as ctx:
        # ---- pools (per-partition bytes in comments; ~208KB available)
        kt_p = ctx.enter_context(tc.tile_pool(name="kt", bufs=1))       # 8 tags x 8KB = 64
        v_p = ctx.enter_context(tc.tile_pool(name="v", bufs=1))         # 16 tags x 2KB = 32 (bf16)
        qt_p = ctx.enter_context(tc.tile_pool(name="qt", bufs=1))       # 8 tags x 4KB = 32
        w_p = ctx.enter_context(tc.tile_pool(name="w", bufs=8))         # 8 x 4KB = 32 (Wk/Wv/Wq/Wo rotate)
        xs_p = ctx.enter_context(tc.tile_pool(name="xs", bufs=16))      # 16 x 1KB = 16
        small_p = ctx.enter_context(tc.tile_pool(name="small", bufs=2)) # maskt 6 + pch 4 + osb 4 = 14
        pt_p = ctx.enter_context(tc.tile_pool(name="pt", bufs=4))       # 4 x 0.25KB = 1 (bf16)
        att_p = ctx.enter_context(tc.tile_pool(name="att", bufs=2))     # 2 x 4KB = 8
        atT_p = ctx.enter_context(tc.tile_pool(name="atT", bufs=10))    # 10 x 0.5KB = 5
        const_p = ctx.enter_context(tc.tile_pool(name="const", bufs=1)) # ~5
        zp_p = ctx.enter_context(tc.tile_pool(name="zp", bufs=4))       # ~0.1
        ps512 = ctx.enter_context(tc.tile_pool(name="ps512", bufs=4, space="PSUM"))
        psatt = ctx.enter_context(tc.tile_pool(name="psatt", bufs=1, space="PSUM"))
        pstr = ctx.enter_context(tc.tile_pool(name="pstr", bufs=2, space="PSUM"))

        # ---- constants
        ident_f = const_p.tile([128, 128], F32, tag="ident_f")
        make_identity(nc, ident_f[:])
        ident = const_p.tile([128, 128], F32R, tag="ident_r")
        nc.scalar.copy(ident[:], ident_f[:])
        bias_bc = const_p.tile([128, D], F32, tag="bias_bc")
        nc.sync.dma_start(bias_bc[:], bbc)
        bq_t = []
        for ec in range(8):
            t = const_p.tile([128, 1], F32, tag=f"bq{ec}", name=f"bqt{ec}")
            nc.sync.dma_start(t[:], bq[ec * 128:(ec + 1) * 128])
            bq_t.append(t)

        def load_w(src):
            ws = []
            for dc in range(8):
                t = w_p.tile([128, D], F32R, tag="wmat", name="wmat")
                nc.sync.dma_start(t[:], src[dc * 128:(dc + 1) * 128, :])
                ws.append(t)
            return ws

        def load_xs(src, c0):
            xsb = []
            for dc in range(8):
                t = xs_p.tile([128, 256], F32R, tag="xs", name="xs")
                nc.sync.dma_start(t[:], src[dc * 128:(dc + 1) * 128, c0:c0 + 256])
                xsb.append(t)
            return xsb

        # ---- K projection: KT[ec] = (x @ Wk)^T rows, [128 e, 2048 s]
        kt = [kt_p.tile([128, S], F32R, tag=f"kt{ec}", name=f"kt{ec}") for ec in range(8)]
        wk_t = load_w(wk)
        for sb in range(8):           # 8 s-blocks of 256
            xsb = load_xs(xt, sb * 256)
            for ec in range(8):
                ps = ps512.tile([128, 512], F32, tag="ps512", name="ps512t")
                for dc in range(8):
                    nc.tensor.matmul(ps[:, 0:256], wk_t[dc][:, ec * 128:(ec + 1) * 128],
                                     xsb[dc][:], start=(dc == 0), stop=(dc == 7))
                nc.vector.tensor_copy(kt[ec][:, sb * 256:(sb + 1) * 256], ps[:, 0:256])

        # ---- V projection (bf16 storage): V[kb] = x @ Wv, [128 s, 1024 e]
        v = [v_p.tile([128, D], BF16, tag=f"v{kb}", name=f"v{kb}") for kb in range(NB)]
        wv_t = load_w(wv)
        for sb in range(8):
            xsb = load_xs(xt, sb * 256)
            for kq in range(2):       # 128-blocks inside the 256 s-block
                kb = sb * 2 + kq
                for eb in range(2):
                    ps = ps512.tile([128, 512], F32, tag="ps512", name="ps512t")
                    for dc in range(8):
                        nc.tensor.matmul(ps[:], xsb[dc][:, kq * 128:(kq + 1) * 128],
                                         wv_t[dc][:, eb * 512:(eb + 1) * 512],
                                         start=(dc == 0), stop=(dc == 7))
                    nc.scalar.copy(v[kb][:, eb * 512:(eb + 1) * 512], ps[:])

        # ---- Q projection (+bq): QT[ec] = (xq @ Wq + bq)^T, [128 e, 1024 q]
        qt = [qt_p.tile([128, QLOC], F32R, tag=f"qt{ec}", name=f"qt{ec}") for ec in range(8)]
        wq_t = load_w(wq)
        for qb in range(4):
            xsb = load_xs(xtq, qb * 256)
            for ec in range(8):
                ps = ps512.tile([128, 512], F32, tag="ps512", name="ps512t")
                for dc in range(8):
                    nc.tensor.matmul(ps[:, 0:256], wq_t[dc][:, ec * 128:(ec + 1) * 128],
                                     xsb[dc][:], start=(dc == 0), stop=(dc == 7))
                nc.scalar.activation(qt[ec][:, qb * 256:(qb + 1) * 256], ps[:, 0:256],
                                     IDENT, bias=bq_t[ec][:])

        # ---- Wo resident (reuses the 8 wmat slots after Wq's last read)
        wo_t = load_w(wo)

        # ---- attention slots
        for j in range(8):
            nkp = NKP[j]
            ch = _chunks(nkp)
            maskt = small_p.tile([128, WIN * 128], F32, tag="maskt", name="maskt", bufs=1)
            nc.sync.dma_start(maskt[:], msk[j, :, :])
            zparts = zp_p.tile([128, 4], F32, tag="zparts", name="zparts")
            att_ps = psatt.tile([128, D], F32, tag="psatt", name="psattt")

            def qk_chunk(ci):
                c0, cw = ch[ci]
                sc = ps512.tile([128, 512], F32, tag="ps512", name="ps512t")
                for ec in range(8):
                    nc.tensor.matmul(sc[:, 0:cw], qt[ec][:, j * 128:(j + 1) * 128],
                                     kt[ec][:, c0:c0 + cw],
                                     start=(ec == 0), stop=(ec == 7))
                for w in range(WIN):
                    boff = (nkp - WIN + w) * 128
                    if c0 <= boff < c0 + cw:
                        nc.vector.tensor_add(sc[:, boff - c0:boff - c0 + 128],
                                             sc[:, boff - c0:boff - c0 + 128],
                                             maskt[:, w * 128:(w + 1) * 128])
                return sc

            def pv_chunk(ci, sc):
                c0, cw = ch[ci]
                pch = small_p.tile([128, 512], F32R, tag="pch", name="pch")
                nc.scalar.activation(pch[:, 0:cw], sc[:, 0:cw], EXP, scale=0.125,
                                     accum_out=zparts[:, ci:ci + 1])
                for bi in range(cw // 128):
                    kb = c0 // 128 + bi
                    tr = pstr.tile([128, 128], F32R, tag="pstr", name="pstrt")
                    nc.tensor.transpose(tr[:], pch[:, bi * 128:(bi + 1) * 128], ident[:])
                    ptb = pt_p.tile([128, 128], BF16, tag="ptb", name="ptb")
                    nc.vector.tensor_copy(ptb[:], tr[:])
                    for eb in range(2):
                        nc.tensor.matmul(att_ps[:, eb * 512:(eb + 1) * 512], ptb[:],
                                         v[kb][:, eb * 512:(eb + 1) * 512],
                                         start=(kb == 0), stop=(kb == nkp - 1))

            # 1-chunk software pipeline: QK(ci+1) is emitted before exp/PV(ci)
            prev = None
            for ci in range(len(ch)):
                sc = qk_chunk(ci)
                if prev is not None:
                    pv_chunk(ci - 1, prev)
                prev = sc
            pv_chunk(len(ch) - 1, prev)

            z = zp_p.tile([128, 1], F32, tag="z", name="zt")
            nc.vector.reduce_sum(z[:], zparts[:, 0:len(ch)], axis=mybir.AxisListType.X)
            rz = zp_p.tile([128, 1], F32, tag="rz", name="rzt")
            nc.vector.reciprocal(rz[:], z[:])
            att_sb = att_p.tile([128, D], F32R, tag="att_sb", name="att_sb")
            nc.scalar.mul(att_sb[:], att_ps[:], rz[:])

            atT = []
            for ec in range(8):
                tr = pstr.tile([128, 128], F32R, tag="pstr", name="pstrt")
                nc.tensor.transpose(tr[:], att_sb[:, ec * 128:(ec + 1) * 128], ident[:])
                t = atT_p.tile([128, 128], F32R, tag="atT", name="atTt")
                nc.scalar.copy(t[:], tr[:])
                atT.append(t)

            for eb in range(2):
                ops = ps512.tile([128, 512], F32, tag="ps512", name="ps512t")
                for ec in range(8):
                    nc.tensor.matmul(ops[:], atT[ec][:], wo_t[ec][:, eb * 512:(eb + 1) * 512],
                                     start=(ec == 0), stop=(ec == 7))
                osb = small_p.tile([128, 512], F32, tag="osb", name="osb")
                nc.vector.tensor_add(osb[:], ops[:], bias_bc[:, eb * 512:(eb + 1) * 512])
                nc.sync.dma_start(outd[j * 128:(j + 1) * 128, eb * 512:(eb + 1) * 512], osb[:])

    nc.compile()
    return nc


def _host_prep(x, Wq, bq, Wk, bk, Wv, bv, Wo, bo):
    """Build the 8 per-core input maps."""
    bop = (bo.astype(np.float64) + bv.astype(np.float64) @ Wo.astype(np.float64)).astype(np.float32)
    bbc = np.ascontiguousarray(np.broadcast_to(bop[None, :], (128, D)))
    tri = np.triu(np.full((128, 128), MASKVAL, np.float32), k=1)  # strictly-above-diag masked
    full = np.full((128, 128), MASKVAL, np.float32)
    zero = np.zeros((128, 128), np.float32)

    in_maps = []
    for core in range(8):
        b = core // 2
        sched = SCHED_A if core % 2 == 0 else SCHED_B
        xtb = np.ascontiguousarray(x[b].T)                       # [D, S]
        xtq = np.ascontiguousarray(
            np.concatenate([xtb[:, g * 128:(g + 1) * 128] for g in sched], axis=1))
        masks = np.empty((8, 128, WIN * 128), np.float32)
        for j, g in enumerate(sched):
            for w in range(WIN):
                kb = NKP[j] - WIN + w
                if kb < g:
                    m = zero
                elif kb == g:
                    m = tri
                else:
                    m = full
                masks[j, :, w * 128:(w + 1) * 128] = m
        in_maps.append({
            "xt": xtb, "xtq": xtq, "wq": Wq, "wk": Wk, "wv": Wv, "wo": Wo,
            "bq": bq, "bbc": bbc, "msk": masks,
        })
    return in_maps


def kernel(x, Wq, bq, Wk, bk, Wv, bv, Wo, bo, _trace=False):
    x = np.asarray(x, np.float32)
    args = [np.asarray(a, np.float32) for a in (Wq, bq, Wk, bk, Wv, bv, Wo, bo)]
    Wq, bq, Wk, bk, Wv, bv, Wo, bo = args

    if "nc" not in _NC_CACHE:
        _NC_CACHE["nc"] = build_nc()
    nc = _NC_CACHE["nc"]

    in_maps = _host_prep(x, Wq, bq, Wk, bk, Wv, bv, Wo, bo)
    res = run_bass_kernel_spmd(nc, in_maps, core_ids=list(range(8)), trace=_trace)
    _NC_CACHE["last_results"] = res

    out = np.empty((B, S, D), np.float32)
    for core in range(8):
        b = core // 2
        sched = SCHED_A if core % 2 == 0 else SCHED_B
        o = res.results[core]["outd"]                            # [QLOC, D]
        for j, g in enumerate(sched):
            out[b, g * 128:(g + 1) * 128, :] = o[j * 128:(j + 1) * 128, :]
    return out
